# revision 3
# baseline (speedup 1.0000x reference)
"""Masked weighted-NLL loss kernel for TRN2 — hand-scheduled engine streams.

reference semantics (B=64, T=188, V=32000, BETA=2.0):
    mask[b,t]   = t < lengths[b]
    gathered    = scores[b, t, gt[b,t]]
    weight[b,t] = 1 if gt[b,t]==0 else BETA
    loss        = -(mask * weight * log(gathered)).sum() / B

Only B*T = 12032 of the 385M score elements are read: each core gathers its
1504 ground-truth scores with indirect DMAs and does log/mask/weight/reduce
on-chip; the host sums the 8 per-core scalars and applies -1/B.

Trace-driven design (53.5us tile-framework baseline -> 34.0us measured):
  * Positions packed [128, 12] (n = 128j + p, n = b*188 + t): the HW SWDGE
    ucode processes exactly ONE offset per partition per indirect-DMA
    instruction (multi-column offset APs gather garbage beyond column 0 on
    real HW), so ceil(1504/128) = 12 instructions is the minimum; each
    costs ~1.1us of serialized Pool time plus ~0.3us dispatch.
  * Offsets reach 48.1M > 2^24, beyond fp32-exact integer range. The DVE
    ALU does int adds in fp32, but the Pool ALU is a true int32 adder
    (HW-verified), so offs = [const table min(n,1503)*V] + gt runs on Pool.
  * Index tables (min(n,1503)*V, n mod 188) are NEFF-const tensors;
    gt/lengths are staged host-side already transposed into the on-chip
    [128, 12] partition-major layout so each input DMA moves 48B-contiguous
    runs per partition instead of 4B/element packets.
  * Synchronization is hand-written with explicit semaphores instead of
    the Tile framework (whose ~40 auto-allocated sems cost a ~10us
    end-of-program reset storm).
  * A DMA's completion semaphore arrives as 16 per-engine increments, and
    engines that moved data straggle ~500-700ns each; the NEFF teardown's
    dma_reset waits for all of them. The [128,1] row-sum output cost ~7us
    of that dribble, so the partition reduce finishes on the PE (ones
    matmul -> PSUM scalar) and the output is a single-descriptor [1,1]
    DMA: 15 of 16 increments arrive instantly.
  * Warm Ln(garbage*0+1) at stream start pulls the 1.3us activation-table
    load off the critical path; mask/weight DVE work overlaps the gathers.
"""

import numpy as np

B, T, V = 64, 188, 32000
N_CORES = 8
B_LOC = B // N_CORES
BETA = 2.0
P = 128
NPOS = B_LOC * T  # 1504
NCOL = (NPOS + P - 1) // P  # 12
NPAD = NCOL * P  # 1536
NELEM = B_LOC * T * V  # 48,128,000

_NC_CACHE = None


def _build_nc():
    import concourse.bacc as bacc
    import concourse.bass as bass
    import concourse.mybir as mybir

    nc = bacc.Bacc("TRN2", target_bir_lowering=False, debug=False)

    scores = nc.dram_tensor("scores", [NELEM, 1], mybir.dt.float32, kind="ExternalInput")
    # meta[p, 0:12] = gt at n=128j+p (0 for pads); meta[p, 12:24] = lengths
    # repeated per position (0 for pads) — staged transposed by the host
    meta = nc.dram_tensor("meta", [P, 2 * NCOL], mybir.dt.int32, kind="ExternalInput")
    out = nc.dram_tensor("out", [1, 1], mybir.dt.float32, kind="ExternalOutput")

    f32 = mybir.dt.float32
    i32 = mybir.dt.int32
    Alu = mybir.AluOpType

    # const index tables, same [128, 24] layout: nv = min(n,1503)*V, tmod = n%T
    n_all = np.arange(NPAD, dtype=np.int64).reshape(NCOL, P).T
    tab_np = np.concatenate(
        [np.minimum(n_all, NPOS - 1) * V, n_all % T], axis=1
    ).astype(np.int32)
    tab = nc.inline_tensor(tab_np, name="idxtab")

    inp = nc.alloc_sbuf_tensor("inp", [P, 2 * NCOL], i32)
    ctab = nc.alloc_sbuf_tensor("ctab", [P, 2 * NCOL], i32)
    offs = nc.alloc_sbuf_tensor("offs", [P, NCOL], i32)
    g = nc.alloc_sbuf_tensor("g", [P, NCOL], f32)
    warm = nc.alloc_sbuf_tensor("warm", [P, 1], f32)
    mask = nc.alloc_sbuf_tensor("mask", [P, NCOL], f32)
    w8 = nc.alloc_sbuf_tensor("w8", [P, NCOL], f32)
    mw = nc.alloc_sbuf_tensor("mw", [P, NCOL], f32)
    logg = nc.alloc_sbuf_tensor("logg", [P, NCOL], f32)
    prod = nc.alloc_sbuf_tensor("prod", [P, NCOL], f32)
    row = nc.alloc_sbuf_tensor("row", [P, 1], f32)
    ones = nc.alloc_sbuf_tensor("ones", [P, 1], f32)
    res = nc.alloc_sbuf_tensor("res", [1, 1], f32)
    tot = nc.alloc_psum_tensor("tot", [1, 1], f32)

    def view(t, col0, ncols):
        a = t[:]
        return bass.AP(a.tensor, col0, [[2 * NCOL, P], [1, ncols]])

    gtN = view(inp, 0, NCOL)
    lenN = view(inp, NCOL, NCOL)
    nv = view(ctab, 0, NCOL)
    tmod = view(ctab, NCOL, NCOL)

    s_in = nc.alloc_semaphore("s_in")
    s_ct = nc.alloc_semaphore("s_ct")
    s_offs = nc.alloc_semaphore("s_offs")
    s_g = nc.alloc_semaphore("s_g")
    s_ln = nc.alloc_semaphore("s_ln")
    s_v = nc.alloc_semaphore("s_v")
    s_row = nc.alloc_semaphore("s_row")
    s_out = nc.alloc_semaphore("s_out")
    s_one = nc.alloc_semaphore("s_one")
    s_mm = nc.alloc_semaphore("s_mm")
    s_res = nc.alloc_semaphore("s_res")

    with nc.Block() as blk:

        @blk.sync
        def _(sync: "bass.BassEngine"):
            sync.dma_start(inp[:], meta[:, :]).then_inc(s_in, 16)
            sync.wait_ge(s_res, 1)
            sync.dma_start(out[:, :], res[:]).then_inc(s_out, 16)
            sync.wait_ge(s_out, 16)

        @blk.scalar
        def _(scalar: "bass.BassEngine"):
            scalar.dma_start(ctab[:], tab[:, :]).then_inc(s_ct, 16)
            # warm activation: Ln(garbage*0 + 1) = 0; pulls the act-table
            # load off the critical path, no input dependency
            scalar.activation(
                warm[:], warm[:], mybir.ActivationFunctionType.Ln,
                bias=1.0, scale=0.0,
            )
            scalar.wait_ge(s_g, 16 * NCOL)
            scalar.activation(
                logg[:], g[:], mybir.ActivationFunctionType.Ln
            ).then_inc(s_ln, 1)

        @blk.gpsimd
        def _(gpsimd: "bass.BassEngine"):
            gpsimd.wait_ge(s_in, 16)
            gpsimd.wait_ge(s_ct, 16)
            gpsimd.tensor_tensor(
                out=offs[:], in0=nv, in1=gtN, op=Alu.add
            ).then_inc(s_offs, 1)
            # the SWDGE desc-gen reads offs asynchronously w.r.t. the ALU
            # write above, so an explicit sem is required even on one engine
            gpsimd.wait_ge(s_offs, 1)
            for j in range(NCOL):
                gpsimd.indirect_dma_start(
                    out=g[:, j : j + 1],
                    out_offset=None,
                    in_=scores[:, :],
                    in_offset=bass.IndirectOffsetOnAxis(
                        ap=offs[:, j : j + 1], axis=0
                    ),
                    element_offset=0,
                ).then_inc(s_g, 16)


        @blk.tensor
        def _(tensor: "bass.BassEngine"):
            tensor.wait_ge(s_one, 1)
            tensor.wait_ge(s_row, 1)
            tensor.matmul(
                tot[:], ones[:], row[:], start=True, stop=True
            ).then_inc(s_mm, 1)

        @blk.vector
        def _(vector: "bass.BassEngine"):
            vector.memset(ones[:], 1.0).then_inc(s_one, 1)
            vector.wait_ge(s_in, 16)
            vector.wait_ge(s_ct, 16)
            vector.tensor_tensor(
                out=mask[:], in0=tmod, in1=lenN, op=Alu.is_lt
            ).then_inc(s_v, 1)
            vector.tensor_scalar(
                out=w8[:], in0=gtN, scalar1=0, scalar2=1,
                op0=Alu.not_equal, op1=Alu.add,
            ).then_inc(s_v, 1)
            vector.wait_ge(s_v, 2)
            vector.tensor_tensor(
                out=mw[:], in0=mask[:], in1=w8[:], op=Alu.mult
            ).then_inc(s_v, 1)
            vector.wait_ge(s_ln, 1)
            vector.wait_ge(s_v, 3)
            vector.tensor_tensor(
                out=prod[:], in0=mw[:], in1=logg[:], op=Alu.mult
            ).then_inc(s_v, 1)
            vector.wait_ge(s_v, 4)
            vector.reduce_sum(
                out=row[:], in_=prod[:], axis=mybir.AxisListType.X
            ).then_inc(s_row, 1)
            vector.wait_ge(s_mm, 1)
            vector.tensor_copy(out=res[:], in_=tot[:]).then_inc(s_res, 1)

    nc.compile()
    return nc


def _shard_inputs(targets_scores, targets_ground_truth, lengths):
    s = np.ascontiguousarray(targets_scores, dtype=np.float32).reshape(
        N_CORES, NELEM, 1
    )
    g_raw = np.ascontiguousarray(targets_ground_truth).astype(np.int32).reshape(
        N_CORES, NPOS
    )
    l_raw = np.repeat(
        np.ascontiguousarray(lengths).astype(np.int32).reshape(N_CORES, B_LOC), T, axis=1
    )
    meta = np.zeros((N_CORES, P, 2 * NCOL), dtype=np.int32)
    for c in range(N_CORES):
        gp = np.zeros(NPAD, dtype=np.int32)
        gp[:NPOS] = g_raw[c]
        lp = np.zeros(NPAD, dtype=np.int32)
        lp[:NPOS] = l_raw[c]
        meta[c, :, :NCOL] = gp.reshape(NCOL, P).T
        meta[c, :, NCOL:] = lp.reshape(NCOL, P).T
    return [{"scores": s[c], "meta": meta[c]} for c in range(N_CORES)]


def _run(targets_scores, targets_ground_truth, lengths, trace=False, **spmd_kwargs):
    from concourse.bass_utils import run_bass_kernel_spmd

    global _NC_CACHE
    if _NC_CACHE is None:
        _NC_CACHE = _build_nc()
    in_maps = _shard_inputs(targets_scores, targets_ground_truth, lengths)
    return run_bass_kernel_spmd(
        _NC_CACHE,
        in_maps,
        core_ids=list(range(N_CORES)),
        trace=trace,
        **spmd_kwargs,
    )


def _finish(results):
    total = np.sum([float(res["out"][0, 0]) for res in results], dtype=np.float64)
    return np.array([-total / B], dtype=np.float32)


def kernel(targets_scores, targets_ground_truth, lengths):
    r = _run(targets_scores, targets_ground_truth, lengths)
    return _finish(r.results)
